# revision 64
# baseline (speedup 1.0000x reference)
"""Multi-head causal attention (B=4, T=2048, E=1024, H=16) on 8 TRN2 NeuronCores.

Sharding: core c handles batch b = c//2 and head-group g = c%2 (8 heads = 512
of the 1024 embedding dims). Each core runs an independent single-core kernel:

  QT  = (Wq_g @ xq.T)   [512, T]  d on partitions, 4 strips of 2 heads each
  KTz = (Wk_g @ xkv.T)  zero-interleaved [128, strip, {hA,hB}, T]: the other
        head's 64 partition rows are zero, so every QK matmul contracts
        K=128. (K<=64 matmuls stream at HALF the PE clock on TRN2; the
        zero-padded K=128 form runs 2x faster despite wasted rows.)
  VE  = (xkv @ Wv_g.T)  [T, 8, 64+ones] bf16 + VE8 fp8e4m3 copy
  per (tq-chunk of 512, head h):
     S.T[tk_blk, tq] = KTz_h[blk].T @ QT  (pairs of 2 blocks -> [128, 1024])
     P.T = exp(S.T/8) * causal_mask
     O.T[65, 512] += [V_h | 1][blk].T @ P.T   (PSUM accumulate)
  O.T (64 rows of sum(p*v) + denominator row) DMAs to DRAM f32; the final
  divide + transpose + unshard happen on the host in numpy.

Precision strategy (target rel_err < 2e-2; achieves ~3.9e-3):
- time slices >= 512 attend over >=512 keys, so quantization noise there
  diffuses away in the softmax average. Their Q/K/V projections run as
  fp8e4m3 DoubleRow matmuls (2 k-tiles per instruction = 2x PE rate),
  while slice 0 stays bf16 (its rows are near-copies of single v rows).
- off-diagonal P.T tiles are fp8; PV runs as one DoubleRow matmul per
  2-block pair (2x). The dA diagonal pair does the same for chunks >= 1
  (block j1 tq-aligned at cols 640:1024; cols 512:640 ride pool buffers
  whose zeros are written once in the prologue). Chunk-0 diagonals and
  all dB pairs stay bf16 with exact exp + triangular masks.
- exp splits across engines: ScalarE does real exp (fp8/bf16 out); every
  4th off pair uses the DVE Schraudolph bit-trick int8(s*8/ln2 + 56)
  whose bits read as fp8e4m3 ~= exp(s). GpSimd applies causal masks.

PSUM layout (8 banks): 3x [128,1024] QK/proj ring + 1x [128,512] dB +
1x [65,512] PV accumulator. Startup DMAs are quartered round-robin over
the three DMA-capable engines (sync/scalar/gpsimd) in consumption order.
"""

import os
import numpy as np
import ml_dtypes

import concourse.bass as bass
import concourse.bacc as bacc
import concourse.mybir as mybir
import concourse.tile as tile
from concourse.bass_utils import run_bass_kernel_spmd

F32 = mybir.dt.float32
BF16 = mybir.dt.bfloat16
F8 = mybir.dt.float8e4
I8 = mybir.dt.int8
DRM = mybir.MatmulPerfMode.DoubleRow
EXP = mybir.ActivationFunctionType.Exp
# Schraudolph fast-exp constants: for scores s (pre-scale raw QK psum),
# int8(s*0.125*8/ln2 + 56) bitcast as fp8e4m3 ~= exp(s*0.125)
FE_A = 0.125 * 8.0 / 0.6931471805599453
FE_B = 56.0

P = 128  # partitions
D = 64  # head dim
B, T_FULL, E, H_TOT = 4, 2048, 1024, 16
HLOC = 8  # heads per core
DLOC = HLOC * D  # 512: local slice of E
N_CORES = 8


def build(T=T_FULL):
    """Single-core graph; same graph runs SPMD on all 8 cores."""
    assert T % 512 == 0
    TC = T // 512  # tq chunks of 512
    NTB = T // P  # tk blocks of 128
    KCH = E // P  # 8 contraction chunks for projections
    MCH = DLOC // P  # 4 output strips for QT/KT

    nc = bacc.Bacc("TRN2", target_bir_lowering=False, debug=False,
                   num_devices=N_CORES)

    xqT = nc.dram_tensor("xqT", [E, 512], BF16, kind="ExternalInput")
    xkvT = nc.dram_tensor("xkvT", [E, 512], BF16, kind="ExternalInput")
    wqT = nc.dram_tensor("wqT", [E, DLOC], BF16, kind="ExternalInput")
    wqT8 = nc.dram_tensor("wqT8", [E, DLOC], F8, kind="ExternalInput")
    wkT8 = nc.dram_tensor("wkT8", [E, DLOC], F8, kind="ExternalInput")
    wvT8 = nc.dram_tensor("wvT8", [E, DLOC], F8, kind="ExternalInput")
    wkT = nc.dram_tensor("wkT", [E, DLOC], BF16, kind="ExternalInput")
    wvT = nc.dram_tensor("wvT", [E, DLOC], BF16, kind="ExternalInput")
    # fp8 copies: time slices >= 512 run their projections in fp8 DoubleRow
    # (2x PE rate); every row there attends over >=512 keys, so the extra
    # quantization noise diffuses away in the softmax average
    xqT8 = nc.dram_tensor("xqT8", [E, T - 512], F8, kind="ExternalInput")
    xkvT8 = nc.dram_tensor("xkvT8", [E, T - 512], F8, kind="ExternalInput")
    # unnormalized O.T per (chunk, head): rows 0:64 sum(p*v), row 64 = sum(p)
    out = nc.dram_tensor("out", [TC, HLOC, D + 1, 512], F32,
                         kind="ExternalOutput")

    xq_v = xqT.ap().rearrange("(k p) t -> p k t", p=P)
    xkv_v = xkvT.ap().rearrange("(k p) t -> p k t", p=P)
    xq8_v = xqT8.ap().rearrange("(k p) t -> p k t", p=P)
    xkv8_v = xkvT8.ap().rearrange("(k p) t -> p k t", p=P)
    wq_v = wqT.ap().rearrange("(k p) d -> p k d", p=P)
    wk_v = wkT.ap().rearrange("(k p) d -> p k d", p=P)
    wv_v = wvT.ap().rearrange("(k p) d -> p k d", p=P)
    w8_vs = {
        "q": wqT8.ap().rearrange("(k p) d -> p k d", p=P),
        "k": wkT8.ap().rearrange("(k p) d -> p k d", p=P),
        "v": wvT8.ap().rearrange("(k p) d -> p k d", p=P),
    }
    out_v = out.ap()

    with tile.TileContext(nc) as tc:
        with (
            tc.tile_pool(name="persist", bufs=1) as persist,
            tc.tile_pool(name="xqpool", bufs=1) as xqpool,
            tc.tile_pool(name="xkpool", bufs=1) as xkpool,
            tc.tile_pool(name="xq8pool", bufs=2) as xq8pool,
            tc.tile_pool(name="xk8pool", bufs=2) as xk8pool,
            tc.tile_pool(name="ptpool", bufs=8) as ptpool,
            tc.tile_pool(name="pt8pool", bufs=14) as pt8pool,
            tc.tile_pool(name="pt8dpool", bufs=4) as pt8dpool,
            tc.tile_pool(name="otpool", bufs=3) as otpool,
            tc.tile_pool(name="big_ps", bufs=3, space="PSUM") as big_ps,
            tc.tile_pool(name="sm_ps", bufs=1, space="PSUM") as sm_ps,
            tc.tile_pool(name="pv_ps", bufs=1, space="PSUM") as pv_ps,
        ):
            # round-robin the three DMA-capable engines so the startup
            # streams interleave in consumption order (~93GB/s per ring)
            ring_engs = [nc.sync, nc.scalar, nc.gpsimd]
            ring_i = [0]

            def ring():
                e = ring_engs[ring_i[0] % 3]
                ring_i[0] += 1
                return e

            wts = {}
            wtiles = {}

            def load_w(nm, src, lo, hi):
                # quarter-tiles on rotating DMA queues: parallel HBM streams
                # and the first projection matmuls wait only for 256KB
                aps = wts.setdefault(nm, [None] * KCH)
                tiles = wtiles.setdefault(nm, [None] * 4)
                for q in range(lo, hi):
                    wt = persist.tile([P, 2, DLOC], BF16, tag=f"w{nm}{q}",
                                      name=f"w{nm}{q}")
                    ring().dma_start(wt[:], src[:, 2 * q : 2 * q + 2, :])
                    aps[2 * q] = wt[:, 0, :]
                    aps[2 * q + 1] = wt[:, 1, :]
                    tiles[q] = wt

            x_tiles = {"q": {}, "kv": {}}

            def get_x(which, n, qlo=0, qhi=4):
                """n=0: bf16 quarter-tiles; n>=1: fp8 half-tiles (two
                DoubleRow k-tile pairs each)."""
                cache = x_tiles[which]
                if n not in cache:
                    cache[n] = [None] * 4
                aps = cache[n]
                c0 = 512 * n
                if n == 0:
                    pool = xqpool if which == "q" else xkpool
                    src = xq_v if which == "q" else xkv_v
                    for q in range(qlo, qhi):
                        if aps[q] is not None:
                            continue
                        xt = pool.tile([P, 2, 512], BF16, tag=f"x{q}",
                                       name=f"x{which}{n}q{q}")
                        ring().dma_start(
                            xt[:], src[:, 2 * q : 2 * q + 2, c0 : c0 + 512])
                        aps[q] = xt
                else:
                    pool = xq8pool if which == "q" else xk8pool
                    src = xq8_v if which == "q" else xkv8_v
                    c0 -= 512
                    for hf in range(2):
                        if aps[2 * hf] is not None:
                            continue
                        xt = pool.tile([P, 2, 2, 512], F8, tag=f"x8{hf}",
                                       name=f"x8{which}{n}h{hf}")
                        eng = ring() if n == 1 else nc.gpsimd
                        eng.dma_start(
                            xt[:],
                            src[:, 4 * hf : 4 * hf + 4, c0 : c0 + 512]
                            .rearrange("p (a b) t -> p a b t", a=2))
                        aps[2 * hf] = xt[:, 0]
                        aps[2 * hf + 1] = xt[:, 1]
                return aps

            w8s = {}

            def load_w8(nm):
                ws = []
                for hf in range(2):
                    wt = persist.tile([P, 2, 2, DLOC], F8, tag=f"w8{nm}{hf}",
                                      name=f"w8{nm}{hf}")
                    ring().dma_start(
                        wt[:],
                        w8_vs[nm][:, 4 * hf : 4 * hf + 4, :]
                        .rearrange("p (a b) d -> p a b d", a=2))
                    ws.append(wt[:, 0])
                    ws.append(wt[:, 1])
                w8s[nm] = ws

            # interleave prologue loads in consumption order:
            # Q units (wq+xq0), K units (wk+xkv0), V units (wv), then the
            # fp8 streams (w8 + x8 slice 1) consumed from chunk 0 onward
            for q in range(4):
                load_w("q", wq_v, q, q + 1)
                get_x("q", 0, q, q + 1)
            for q in range(4):
                load_w("k", wk_v, q, q + 1)
                get_x("kv", 0, q, q + 1)
            load_w("v", wv_v, 0, 4)
            load_w8("q")
            load_w8("k")
            load_w8("v")
            get_x("q", 1)
            get_x("kv", 1)

            # ---- constants ----
            # tri2[:, a, :]: upper triangle (keep col >= row), for the two
            # 128-wide diagonal sub-blocks handled per mask op
            tri2 = persist.tile([P, 2, P], BF16, tag="tri2")
            nc.gpsimd.memset(tri2[:], 1.0)
            for a in range(2):
                nc.gpsimd.affine_select(
                    out=tri2[:, a, :],
                    in_=tri2[:, a, :],
                    compare_op=mybir.AluOpType.is_ge,
                    fill=0.0,
                    base=0,
                    pattern=[[1, P]],
                    channel_multiplier=-1,
                )

            QT = persist.tile([P, MCH, T], BF16, tag="QT")
            # KTz: zero-interleaved K layout. KTz[:, spo, 0, :] has head
            # 2*spo rows on partitions 0:64 and ZEROS on 64:128;
            # KTz[:, spo, 1, :] the reverse. QK matmuls then use the full
            # 128-partition contraction: K<=64 matmuls stream at half the
            # PE clock, so the zero-padded K=128 form runs 2x faster.
            KTz = persist.tile([P, MCH, 2, T], BF16, tag="KTz")
            nc.vector.memset(KTz[0 : P // 2, :, 1, :], 0.0)
            nc.gpsimd.memset(KTz[P // 2 : P, :, 0, :], 0.0)
            VE = persist.tile([P, NTB, HLOC, D + 1], BF16, tag="VE")
            nc.vector.memset(VE[:, :, :, D : D + 1], 1.0)
            # fp8 copy of V (+ones) for the off-diagonal DoubleRow PV path.
            # Padded to D+2 so the k-tile stride (8*66=528B) meets the
            # DoubleRow LDWEIGHTS 16B stride-alignment ISA rule; the pad
            # column is never read (lhsT slices [..., 0:D+1]).
            VE8 = persist.tile([P, NTB, HLOC, D + 2], F8, tag="VE8")
            nc.gpsimd.memset(VE8[:, :, :, D : D + 1], 1.0)
            # pre-zero cols [512:640) of the 4 pt8d ring buffers once: the
            # fp8 diagonal pairs never write that range, so the zeros
            # persist across ring reuse (block j1 is tq-aligned at 640:1024)
            for zi in range(4):
                zt = pt8dpool.tile([P, 1024], F8, tag="pt8d", name=f"z8{zi}")
                nc.gpsimd.memset(zt[:, 512:640], 0.0)

            # ---- projection units (8 bf16 / 4 fp8-DR matmuls + casts) ----
            def unit_qk(nm, n, m):
                xt = get_x("q" if nm == "q" else "kv", n)
                ps = big_ps.tile([P, 1024], F32, tag="big", name="pjps")
                if n == 0:
                    wt = wts[nm]
                    for k in range(KCH):
                        nc.tensor.matmul(
                            ps[:, 0:512],
                            wt[k][:, P * m : P * m + P],
                            xt[k // 2][:, k % 2, :],
                            start=(k == 0),
                            stop=(k == KCH - 1),
                        )
                else:
                    wt = w8s[nm]
                    for k2 in range(KCH // 2):
                        nc.tensor.matmul(
                            ps[:, 0:512],
                            wt[k2][:, :, P * m : P * m + P],
                            xt[k2],
                            start=(k2 == 0),
                            stop=(k2 == KCH // 2 - 1),
                            perf_mode=DRM,
                        )
                c0 = 512 * n
                if nm == "q":
                    nc.vector.tensor_copy(QT[:, m, c0 : c0 + 512],
                                          ps[:, 0:512])
                else:
                    # split cast into the zero-interleaved KTz layout
                    nc.vector.tensor_copy(
                        KTz[0 : P // 2, m, 0, c0 : c0 + 512],
                        ps[0 : P // 2, 0:512])
                    nc.scalar.copy(
                        KTz[P // 2 : P, m, 1, c0 : c0 + 512],
                        ps[P // 2 : P, 0:512])

            def unit_v(n, r):
                i = 4 * n + r
                xt = get_x("kv", n)
                ps = big_ps.tile([P, 1024], F32, tag="big", name="pvps")
                if n == 0:
                    wt = wts["v"]
                    for k in range(KCH):
                        nc.tensor.matmul(
                            ps[:, 0:512],
                            xt[k // 2][:, k % 2, P * r : P * r + P],
                            wt[k],
                            start=(k == 0),
                            stop=(k == KCH - 1),
                        )
                else:
                    wt = w8s["v"]
                    for k2 in range(KCH // 2):
                        nc.tensor.matmul(
                            ps[:, 0:512],
                            xt[k2][:, :, P * r : P * r + P],
                            wt[k2],
                            start=(k2 == 0),
                            stop=(k2 == KCH // 2 - 1),
                            perf_mode=DRM,
                        )
                nc.vector.tensor_copy(
                    VE[:, i, :, 0:D],
                    ps[:, 0:512].rearrange("p (h d) -> p h d", h=HLOC),
                )
                nc.vector.tensor_copy(VE8[:, i, :, 0:D], VE[:, i, :, 0:D])

            # ---- attention pieces ----
            def emit_qk_pair(c, h, kind, pidx):
                """Emit one QK pair: 2 matmuls -> 1 exp -> optional mask.

                Returns pv entries: ("dr", pt8, j0) for fp8 DoubleRow pairs
                or ("mm", pt, j, off, st, w) for bf16 diagonal blocks.
                """
                spo, sel = h // 2, h % 2
                q0 = 512 * c
                ps = big_ps.tile([P, 1024], F32, tag="big", name="qps")
                if kind == "off":
                    j0 = 2 * pidx
                    for t2 in range(2):
                        j = j0 + t2
                        nc.tensor.matmul(
                            ps[:, 512 * t2 : 512 * t2 + 512],
                            KTz[:, spo, sel, P * j : P * j + P],
                            QT[:, spo, q0 : q0 + 512],
                            start=True,
                            stop=True,
                        )
                    pt8 = pt8pool.tile([P, 1024], F8, tag="pt8", name="pt8")
                    if pidx % 4 == 1:
                        # fast-exp on DVE: y = s*FE_A + FE_B converted to
                        # int8, whose bits read as fp8e4m3 give ~exp(s/8)
                        nc.vector.tensor_scalar(
                            pt8[:].bitcast(I8), ps[:], FE_A, FE_B,
                            mybir.AluOpType.mult, mybir.AluOpType.add)
                    else:
                        nc.scalar.activation(pt8[:], ps[:], EXP, scale=0.125)
                    return [("dr", pt8, j0)]
                if kind == "dA" and c >= 1:
                    # fp8 DoubleRow dA (rows attend >=512 keys): j1 sits
                    # tq-aligned at cols 640:1024; 512:640 stay pool-zeroed
                    j0, j1 = 4 * c, 4 * c + 1
                    nc.tensor.matmul(
                        ps[:, 0:512],
                        KTz[:, spo, sel, P * j0 : P * j0 + P],
                        QT[:, spo, q0 : q0 + 512],
                        start=True,
                        stop=True,
                    )
                    nc.tensor.matmul(
                        ps[:, 640:1024],
                        KTz[:, spo, sel, P * j1 : P * j1 + P],
                        QT[:, spo, q0 + 128 : q0 + 512],
                        start=True,
                        stop=True,
                    )
                    pt8 = pt8dpool.tile([P, 1024], F8, tag="pt8d",
                                        name="pt8d")
                    nc.scalar.activation(pt8[:, 0:512], ps[:, 0:512], EXP,
                                         scale=0.125)
                    nc.scalar.activation(pt8[:, 640:1024], ps[:, 640:1024],
                                         EXP, scale=0.125)
                    masks.append(("aff8", pt8[:, 0:P]))
                    masks.append(("aff8", pt8[:, 640 : 640 + P]))
                    return [("dr", pt8, j0)]
                pt = ptpool.tile([P, 1024], BF16, tag="pt", name="pt")
                if kind == "dA":
                    j0, j1 = 4 * c, 4 * c + 1
                    nc.tensor.matmul(
                        ps[:, 0:512],
                        KTz[:, spo, sel, P * j0 : P * j0 + P],
                        QT[:, spo, q0 : q0 + 512],
                        start=True,
                        stop=True,
                    )
                    nc.tensor.matmul(
                        ps[:, 512:896],
                        KTz[:, spo, sel, P * j1 : P * j1 + P],
                        QT[:, spo, q0 + 128 : q0 + 512],
                        start=True,
                        stop=True,
                    )
                    nc.scalar.activation(pt[:, 0:896], ps[:, 0:896], EXP,
                                         scale=0.125)
                    # mask regions: block j0 cols [0,128) at off 0; block j1
                    # cols [128,256) at off 512+0
                    mv = pt[:].rearrange("p (a b) -> p a b", a=2)[:, :, 0:P]
                    masks.append(("mul", mv, tri2[:]))
                    return [("mm", pt, j0, 0, 0, 512),
                            ("mm", pt, j1, 512, 128, 384)]
                # dB
                j2, j3 = 4 * c + 2, 4 * c + 3
                ps = sm_ps.tile([P, 512], F32, tag="sm", name="dbps")
                nc.tensor.matmul(
                    ps[:, 0:256],
                    KTz[:, spo, sel, P * j2 : P * j2 + P],
                    QT[:, spo, q0 + 256 : q0 + 512],
                    start=True,
                    stop=True,
                )
                nc.tensor.matmul(
                    ps[:, 256:384],
                    KTz[:, spo, sel, P * j3 : P * j3 + P],
                    QT[:, spo, q0 + 384 : q0 + 512],
                    start=True,
                    stop=True,
                )
                nc.scalar.activation(pt[:, 0:384], ps[:, 0:384], EXP,
                                     scale=0.125)
                # mask regions: block j2 cols [256,384) at off 0; block j3
                # cols [384,512) at off 256
                mv = pt[:].rearrange("p (a b) -> p a b", a=4)[:, 0:2, 0:P]
                masks.append(("mul", mv, tri2[:]))
                return [("mm", pt, j2, 0, 256, 256),
                        ("mm", pt, j3, 256, 384, 128)]

            # ---- per-step state ----
            pend_pv = None  # (c, h, entries) awaiting PV in the next step
            masks = []  # deferred diagonal mask multiplies

            class PvStepper:
                """Emits the PV accumulation for one (c,h) step, one entry at
                a time, so the matmuls can interleave between the next step's
                QK pairs (keeping the PE fed while ScalarE/DVE drain exps)."""

                def __init__(self, c, h, entries):
                    self.c, self.h, self.entries = c, h, entries
                    self.i = 0
                    self.pv = pv_ps.tile([D + 1, 512], F32, tag="pv",
                                         name="pv")

                def step(self, k=1):
                    h, n = self.h, len(self.entries)
                    while k > 0 and self.i < n:
                        e = self.entries[self.i]
                        if e[0] == "dr":
                            _, pt8, j = e
                            nc.tensor.matmul(
                                self.pv[:, 0:512],
                                VE8[:, j : j + 2, h, 0 : D + 1],
                                pt8[:].rearrange("p (a n) -> p a n", a=2),
                                start=(self.i == 0),
                                stop=(self.i == n - 1),
                                perf_mode=DRM,
                                skip_group_check=True,
                            )
                        else:
                            _, pt, j, off, st, w = e
                            nc.tensor.matmul(
                                self.pv[:, st : st + w],
                                VE[:, j, h, :],
                                pt[:, off : off + w],
                                start=(self.i == 0),
                                stop=(self.i == n - 1),
                                skip_group_check=True,
                            )
                        self.i += 1
                        k -= 1

                def finish(self, units, quota, split=False):
                    n = len(self.entries)
                    while self.i < n:
                        if self.i % 2 == 1 and units and quota:
                            units.pop(0)()
                            quota -= 1
                        self.step()
                    while units and quota:
                        units.pop(0)()
                        quota -= 1
                    ot = otpool.tile([D + 1, 512], F32, tag="ot", name="ot")
                    if split:
                        # tail: stream the first half while casting the rest
                        nc.vector.tensor_copy(ot[:, 0:256], self.pv[:, 0:256])
                        nc.sync.dma_start(out_v[self.c, self.h, :, 0:256],
                                          ot[:, 0:256])
                        nc.vector.tensor_copy(ot[:, 256:512],
                                              self.pv[:, 256:512])
                        nc.sync.dma_start(out_v[self.c, self.h, :, 256:512],
                                          ot[:, 256:512])
                    else:
                        nc.vector.tensor_copy(ot[:, 0:256],
                                              self.pv[:, 0:256])
                        nc.scalar.copy(ot[:, 256:512], self.pv[:, 256:512])
                        nc.sync.dma_start(out_v[self.c, self.h], ot[:])

            # ---- prologue: slice-0 projections ----
            for m in range(MCH):
                unit_qk("q", 0, m)
            for m in range(MCH):
                unit_qk("k", 0, m)
            for r in range(4):
                unit_v(0, r)

            # ---- main steps ----
            units = []
            for c in range(TC):
                if c + 1 < TC:
                    for m in range(MCH):
                        units.append(
                            lambda m=m, n=c + 1: unit_qk("q", n, m))
                        units.append(
                            lambda m=m, n=c + 1: unit_qk("k", n, m))
                    for r in range(4):
                        units.append(lambda r=r, n=c + 1: unit_v(n, r))
                for h in range(HLOC):
                    if c == TC - 1:
                        quota = 0
                    else:
                        quota = len(units) if h == HLOC - 1 else 2
                    kinds = [("off", p) for p in range(2 * c)]
                    kinds += [("dA", 0), ("dB", 0)]
                    stepper = PvStepper(*pend_pv) if pend_pv else None
                    entries = []
                    for np_, (kind, pidx) in enumerate(kinds):
                        entries += emit_qk_pair(c, h, kind, pidx)
                        if np_ == 1 and units and quota:
                            units.pop(0)()
                            quota -= 1
                    if stepper:
                        stepper.finish(units, quota)
                    # deferred diagonal masks flush after the PV cast (on
                    # gpsimd, which is otherwise idle mid-kernel)
                    for item in masks:
                        if item[0] == "mul":
                            nc.gpsimd.tensor_mul(item[1], item[1], item[2])
                        else:
                            nc.gpsimd.affine_select(
                                out=item[1], in_=item[1],
                                compare_op=mybir.AluOpType.is_ge,
                                fill=0.0, base=0, pattern=[[1, P]],
                                channel_multiplier=-1)
                    masks.clear()
                    pend_pv = (c, h, entries)

            # ---- drain ----
            st_ = PvStepper(*pend_pv)
            st_.finish([], 0, split=True)

    nc.compile()
    return nc


_NC_CACHE = {}


def _get_nc(T):
    if T not in _NC_CACHE:
        _NC_CACHE[T] = build(T)
    return _NC_CACHE[T]


def kernel(inputs_q, inputs_kv, Wq, Wk, Wv):
    inputs_q = np.asarray(inputs_q, dtype=np.float32)
    inputs_kv = np.asarray(inputs_kv, dtype=np.float32)
    Wq = np.asarray(Wq, dtype=np.float32)
    Wk = np.asarray(Wk, dtype=np.float32)
    Wv = np.asarray(Wv, dtype=np.float32)
    T = inputs_q.shape[1]
    TC = T // 512

    bf = ml_dtypes.bfloat16
    f8 = ml_dtypes.float8_e4m3
    in_maps = []
    for c in range(N_CORES):
        b, g = c // 2, c % 2
        sl = slice(g * DLOC, (g + 1) * DLOC)
        xqt = np.ascontiguousarray(inputs_q[b].T)
        xkvt = np.ascontiguousarray(inputs_kv[b].T)
        in_maps.append(
            {
                "xqT": xqt[:, 0:512].astype(bf),
                "xkvT": xkvt[:, 0:512].astype(bf),
                "xqT8": xqt[:, 512:].astype(f8),
                "xkvT8": xkvt[:, 512:].astype(f8),
                "wqT": np.ascontiguousarray(Wq[sl].T).astype(bf),
                "wkT": np.ascontiguousarray(Wk[sl].T).astype(bf),
                "wvT": np.ascontiguousarray(Wv[sl].T).astype(bf),
                "wqT8": np.ascontiguousarray(Wq[sl].T).astype(f8),
                "wkT8": np.ascontiguousarray(Wk[sl].T).astype(f8),
                "wvT8": np.ascontiguousarray(Wv[sl].T).astype(f8),
            }
        )

    nc = _get_nc(T)
    trace = bool(int(os.environ.get("KERNEL_TRACE", "0")))
    res = run_bass_kernel_spmd(
        nc, in_maps, core_ids=list(range(N_CORES)), trace=trace
    )
    if trace:
        kernel.last_result = res

    full = np.empty((B, T, E), np.float32)
    for c in range(N_CORES):
        b, g = c // 2, c % 2
        ot = res.results[c]["out"]  # [TC, HLOC, D+1, 512]
        o = ot[:, :, 0:D, :] / ot[:, :, D : D + 1, :]
        # [TC, HLOC, D, 512] -> [TC, 512, HLOC, D] -> [T, DLOC]
        o = o.transpose(0, 3, 1, 2).reshape(T, DLOC)
        full[b, :, g * DLOC : (g + 1) * DLOC] = o
    return full


# revision 65
# speedup vs baseline: 1.0710x; 1.0710x over previous
"""Multi-head causal attention (B=4, T=2048, E=1024, H=16) on 8 TRN2 NeuronCores.

Sharding: core c handles batch b = c//2 and head-group g = c%2 (8 heads = 512
of the 1024 embedding dims). Each core runs an independent single-core kernel:

  QT  = (Wq_g @ xq.T)   [512, T]  d on partitions, 4 strips of 2 heads each
  KTz = (Wk_g @ xkv.T)  zero-interleaved [128, strip, {hA,hB}, T]: the other
        head's 64 partition rows are zero, so every QK matmul contracts
        K=128. (K<=64 matmuls stream at HALF the PE clock on TRN2; the
        zero-padded K=128 form runs 2x faster despite wasted rows.)
  VE  = (xkv @ Wv_g.T)  [T, 8, 64+ones] bf16 + VE8 fp8e4m3 copy
  per (tq-chunk of 512, head h):
     S.T[tk_blk, tq] = KTz_h[blk].T @ QT  (pairs of 2 blocks -> [128, 1024])
     P.T = exp(S.T/8) * causal_mask
     O.T[65, 512] += [V_h | 1][blk].T @ P.T   (PSUM accumulate)
  O.T (64 rows of sum(p*v) + denominator row) DMAs to DRAM f32; the final
  divide + transpose + unshard happen on the host in numpy.

Precision strategy (target rel_err < 2e-2; achieves ~3.9e-3):
- time slices >= 512 attend over >=512 keys, so quantization noise there
  diffuses away in the softmax average. Their Q/K/V projections run as
  fp8e4m3 DoubleRow matmuls (2 k-tiles per instruction = 2x PE rate),
  while slice 0 stays bf16 (its rows are near-copies of single v rows).
- off-diagonal P.T tiles are fp8; PV runs as one DoubleRow matmul per
  2-block pair (2x). The dA diagonal pair does the same for chunks >= 1
  (block j1 tq-aligned at cols 640:1024; cols 512:640 ride pool buffers
  whose zeros are written once in the prologue). Chunk-0 diagonals and
  all dB pairs stay bf16 with exact exp + triangular masks.
- exp splits across engines: ScalarE does real exp (fp8/bf16 out); every
  4th off pair uses the DVE Schraudolph bit-trick int8(s*8/ln2 + 56)
  whose bits read as fp8e4m3 ~= exp(s). GpSimd applies causal masks.

PSUM layout (8 banks): 3x [128,1024] QK/proj ring + 1x [128,512] dB +
1x [65,512] PV accumulator. Startup DMAs are quartered round-robin over
the three DMA-capable engines (sync/scalar/gpsimd) in consumption order.
"""

import os
import numpy as np
import ml_dtypes

import concourse.bass as bass
import concourse.bacc as bacc
import concourse.mybir as mybir
import concourse.tile as tile
from concourse.bass_utils import run_bass_kernel_spmd

F32 = mybir.dt.float32
BF16 = mybir.dt.bfloat16
F8 = mybir.dt.float8e4
I8 = mybir.dt.int8
DRM = mybir.MatmulPerfMode.DoubleRow
EXP = mybir.ActivationFunctionType.Exp
# Schraudolph fast-exp constants: for scores s (pre-scale raw QK psum),
# int8(s*0.125*8/ln2 + 56) bitcast as fp8e4m3 ~= exp(s*0.125)
FE_A = 0.125 * 8.0 / 0.6931471805599453
FE_B = 56.0

P = 128  # partitions
D = 64  # head dim
B, T_FULL, E, H_TOT = 4, 2048, 1024, 16
HLOC = 8  # heads per core
DLOC = HLOC * D  # 512: local slice of E
N_CORES = 8


def build(T=T_FULL):
    """Single-core graph; same graph runs SPMD on all 8 cores."""
    assert T % 512 == 0
    TC = T // 512  # tq chunks of 512
    NTB = T // P  # tk blocks of 128
    KCH = E // P  # 8 contraction chunks for projections
    MCH = DLOC // P  # 4 output strips for QT/KT

    nc = bacc.Bacc("TRN2", target_bir_lowering=False, debug=False,
                   num_devices=N_CORES)

    xqT = nc.dram_tensor("xqT", [E, 512], BF16, kind="ExternalInput")
    xkvT = nc.dram_tensor("xkvT", [E, 512], BF16, kind="ExternalInput")
    wqT = nc.dram_tensor("wqT", [E, DLOC], BF16, kind="ExternalInput")
    wqT8 = nc.dram_tensor("wqT8", [E, DLOC], F8, kind="ExternalInput")
    wkT8 = nc.dram_tensor("wkT8", [E, DLOC], F8, kind="ExternalInput")
    wvT8 = nc.dram_tensor("wvT8", [E, DLOC], F8, kind="ExternalInput")
    wkT = nc.dram_tensor("wkT", [E, DLOC], BF16, kind="ExternalInput")
    wvT = nc.dram_tensor("wvT", [E, DLOC], BF16, kind="ExternalInput")
    # fp8 copies: time slices >= 512 run their projections in fp8 DoubleRow
    # (2x PE rate); every row there attends over >=512 keys, so the extra
    # quantization noise diffuses away in the softmax average
    xqT8 = nc.dram_tensor("xqT8", [E, T - 512], F8, kind="ExternalInput")
    xkvT8 = nc.dram_tensor("xkvT8", [E, T - 512], F8, kind="ExternalInput")
    # unnormalized O.T per (chunk, head): rows 0:64 sum(p*v), row 64 = sum(p)
    out = nc.dram_tensor("out", [TC, HLOC, D + 1, 512], F32,
                         kind="ExternalOutput")

    xq_v = xqT.ap().rearrange("(k p) t -> p k t", p=P)
    xkv_v = xkvT.ap().rearrange("(k p) t -> p k t", p=P)
    xq8_v = xqT8.ap().rearrange("(k p) t -> p k t", p=P)
    xkv8_v = xkvT8.ap().rearrange("(k p) t -> p k t", p=P)
    wq_v = wqT.ap().rearrange("(k p) d -> p k d", p=P)
    wk_v = wkT.ap().rearrange("(k p) d -> p k d", p=P)
    wv_v = wvT.ap().rearrange("(k p) d -> p k d", p=P)
    w8_vs = {
        "q": wqT8.ap().rearrange("(k p) d -> p k d", p=P),
        "k": wkT8.ap().rearrange("(k p) d -> p k d", p=P),
        "v": wvT8.ap().rearrange("(k p) d -> p k d", p=P),
    }
    out_v = out.ap()

    with tile.TileContext(nc) as tc:
        with (
            tc.tile_pool(name="persist", bufs=1) as persist,
            tc.tile_pool(name="xqpool", bufs=1) as xqpool,
            tc.tile_pool(name="xkpool", bufs=1) as xkpool,
            tc.tile_pool(name="xq8pool", bufs=2) as xq8pool,
            tc.tile_pool(name="xk8pool", bufs=2) as xk8pool,
            tc.tile_pool(name="ptpool", bufs=8) as ptpool,
            tc.tile_pool(name="pt8pool", bufs=14) as pt8pool,
            tc.tile_pool(name="pt8dpool", bufs=4) as pt8dpool,
            tc.tile_pool(name="otpool", bufs=3) as otpool,
            tc.tile_pool(name="big_ps", bufs=3, space="PSUM") as big_ps,
            tc.tile_pool(name="sm_ps", bufs=1, space="PSUM") as sm_ps,
            tc.tile_pool(name="pv_ps", bufs=1, space="PSUM") as pv_ps,
        ):
            # round-robin the three DMA-capable engines so the startup
            # streams interleave in consumption order (~93GB/s per ring)
            ring_engs = [nc.sync, nc.scalar, nc.gpsimd]
            ring_i = [0]

            def ring():
                e = ring_engs[ring_i[0] % 3]
                ring_i[0] += 1
                return e

            wts = {}
            wtiles = {}

            def load_w(nm, src, lo, hi):
                # quarter-tiles on rotating DMA queues: parallel HBM streams
                # and the first projection matmuls wait only for 256KB
                aps = wts.setdefault(nm, [None] * KCH)
                tiles = wtiles.setdefault(nm, [None] * 4)
                for q in range(lo, hi):
                    wt = persist.tile([P, 2, DLOC], BF16, tag=f"w{nm}{q}",
                                      name=f"w{nm}{q}")
                    ring().dma_start(wt[:], src[:, 2 * q : 2 * q + 2, :])
                    aps[2 * q] = wt[:, 0, :]
                    aps[2 * q + 1] = wt[:, 1, :]
                    tiles[q] = wt

            x_tiles = {"q": {}, "kv": {}}

            def get_x(which, n, qlo=0, qhi=4):
                """n=0: bf16 quarter-tiles; n>=1: fp8 half-tiles (two
                DoubleRow k-tile pairs each)."""
                cache = x_tiles[which]
                if n not in cache:
                    cache[n] = [None] * 4
                aps = cache[n]
                c0 = 512 * n
                if n == 0:
                    pool = xqpool if which == "q" else xkpool
                    src = xq_v if which == "q" else xkv_v
                    for q in range(qlo, qhi):
                        if aps[q] is not None:
                            continue
                        xt = pool.tile([P, 2, 512], BF16, tag=f"x{q}",
                                       name=f"x{which}{n}q{q}")
                        ring().dma_start(
                            xt[:], src[:, 2 * q : 2 * q + 2, c0 : c0 + 512])
                        aps[q] = xt
                else:
                    pool = xq8pool if which == "q" else xk8pool
                    src = xq8_v if which == "q" else xkv8_v
                    c0 -= 512
                    for hf in range(2):
                        if aps[2 * hf] is not None:
                            continue
                        xt = pool.tile([P, 2, 2, 512], F8, tag=f"x8{hf}",
                                       name=f"x8{which}{n}h{hf}")
                        eng = ring() if n == 1 else nc.gpsimd
                        eng.dma_start(
                            xt[:],
                            src[:, 4 * hf : 4 * hf + 4, c0 : c0 + 512]
                            .rearrange("p (a b) t -> p a b t", a=2))
                        aps[2 * hf] = xt[:, 0]
                        aps[2 * hf + 1] = xt[:, 1]
                return aps

            w8s = {}

            def load_w8(nm):
                ws = []
                for hf in range(2):
                    wt = persist.tile([P, 2, 2, DLOC], F8, tag=f"w8{nm}{hf}",
                                      name=f"w8{nm}{hf}")
                    ring().dma_start(
                        wt[:],
                        w8_vs[nm][:, 4 * hf : 4 * hf + 4, :]
                        .rearrange("p (a b) d -> p a b d", a=2))
                    ws.append(wt[:, 0])
                    ws.append(wt[:, 1])
                w8s[nm] = ws

            # interleave prologue loads in consumption order:
            # Q units (wq+xq0), K units (wk+xkv0), V units (wv), then the
            # fp8 streams (w8 + x8 slice 1) consumed from chunk 0 onward
            for q in range(4):
                load_w("q", wq_v, q, q + 1)
                get_x("q", 0, q, q + 1)
            for q in range(4):
                load_w("k", wk_v, q, q + 1)
                get_x("kv", 0, q, q + 1)
            load_w("v", wv_v, 0, 4)
            load_w8("q")
            load_w8("k")
            load_w8("v")
            get_x("q", 1)
            get_x("kv", 1)

            # ---- constants ----
            # tri2[:, a, :]: upper triangle (keep col >= row), for the two
            # 128-wide diagonal sub-blocks handled per mask op
            tri2 = persist.tile([P, 2, P], BF16, tag="tri2")
            nc.gpsimd.memset(tri2[:], 1.0)
            for a in range(2):
                nc.gpsimd.affine_select(
                    out=tri2[:, a, :],
                    in_=tri2[:, a, :],
                    compare_op=mybir.AluOpType.is_ge,
                    fill=0.0,
                    base=0,
                    pattern=[[1, P]],
                    channel_multiplier=-1,
                )

            QT = persist.tile([P, MCH, T], BF16, tag="QT")
            # KTz: zero-interleaved K layout. KTz[:, spo, 0, :] has head
            # 2*spo rows on partitions 0:64 and ZEROS on 64:128;
            # KTz[:, spo, 1, :] the reverse. QK matmuls then use the full
            # 128-partition contraction: K<=64 matmuls stream at half the
            # PE clock, so the zero-padded K=128 form runs 2x faster.
            KTz = persist.tile([P, MCH, 2, T], BF16, tag="KTz")
            nc.vector.memset(KTz[0 : P // 2, :, 1, :], 0.0)
            nc.gpsimd.memset(KTz[P // 2 : P, :, 0, :], 0.0)
            VE = persist.tile([P, NTB, HLOC, D + 1], BF16, tag="VE")
            nc.vector.memset(VE[:, :, :, D : D + 1], 1.0)
            # fp8 copy of V (+ones) for the off-diagonal DoubleRow PV path.
            # Padded to D+2 so the k-tile stride (8*66=528B) meets the
            # DoubleRow LDWEIGHTS 16B stride-alignment ISA rule; the pad
            # column is never read (lhsT slices [..., 0:D+1]).
            VE8 = persist.tile([P, NTB, HLOC, D + 2], F8, tag="VE8")
            nc.gpsimd.memset(VE8[:, :, :, D : D + 1], 1.0)
            # pre-zero cols [512:640) of the 4 pt8d ring buffers once: the
            # fp8 diagonal pairs never write that range, so the zeros
            # persist across ring reuse (block j1 is tq-aligned at 640:1024)
            for zi in range(4):
                zt = pt8dpool.tile([P, 1024], F8, tag="pt8d", name=f"z8{zi}")
                nc.gpsimd.memset(zt[:, 512:640], 0.0)

            # ---- projection units (8 bf16 / 4 fp8-DR matmuls + casts) ----
            def unit_qk(nm, n, m):
                xt = get_x("q" if nm == "q" else "kv", n)
                ps = big_ps.tile([P, 1024], F32, tag="big", name="pjps")
                if n == 0:
                    wt = wts[nm]
                    for k in range(KCH):
                        nc.tensor.matmul(
                            ps[:, 0:512],
                            wt[k][:, P * m : P * m + P],
                            xt[k // 2][:, k % 2, :],
                            start=(k == 0),
                            stop=(k == KCH - 1),
                        )
                else:
                    wt = w8s[nm]
                    for k2 in range(KCH // 2):
                        nc.tensor.matmul(
                            ps[:, 0:512],
                            wt[k2][:, :, P * m : P * m + P],
                            xt[k2],
                            start=(k2 == 0),
                            stop=(k2 == KCH // 2 - 1),
                            perf_mode=DRM,
                        )
                c0 = 512 * n
                if nm == "q":
                    nc.vector.tensor_copy(QT[:, m, c0 : c0 + 512],
                                          ps[:, 0:512])
                else:
                    # split cast into the zero-interleaved KTz layout
                    nc.vector.tensor_copy(
                        KTz[0 : P // 2, m, 0, c0 : c0 + 512],
                        ps[0 : P // 2, 0:512])
                    nc.scalar.copy(
                        KTz[P // 2 : P, m, 1, c0 : c0 + 512],
                        ps[P // 2 : P, 0:512])

            def unit_v(n, r):
                i = 4 * n + r
                xt = get_x("kv", n)
                ps = big_ps.tile([P, 1024], F32, tag="big", name="pvps")
                if n == 0:
                    wt = wts["v"]
                    for k in range(KCH):
                        nc.tensor.matmul(
                            ps[:, 0:512],
                            xt[k // 2][:, k % 2, P * r : P * r + P],
                            wt[k],
                            start=(k == 0),
                            stop=(k == KCH - 1),
                        )
                else:
                    wt = w8s["v"]
                    for k2 in range(KCH // 2):
                        nc.tensor.matmul(
                            ps[:, 0:512],
                            xt[k2][:, :, P * r : P * r + P],
                            wt[k2],
                            start=(k2 == 0),
                            stop=(k2 == KCH // 2 - 1),
                            perf_mode=DRM,
                        )
                nc.vector.tensor_copy(
                    VE[:, i, :, 0:D],
                    ps[:, 0:512].rearrange("p (h d) -> p h d", h=HLOC),
                )
                nc.vector.tensor_copy(VE8[:, i, :, 0:D], VE[:, i, :, 0:D])

            # ---- attention pieces ----
            def emit_qk_pair(c, h, kind, pidx):
                """Emit one QK pair: 2 matmuls -> 1 exp -> optional mask.

                Returns pv entries: ("dr", pt8, j0) for fp8 DoubleRow pairs
                or ("mm", pt, j, off, st, w) for bf16 diagonal blocks.
                """
                spo, sel = h // 2, h % 2
                q0 = 512 * c
                ps = big_ps.tile([P, 1024], F32, tag="big", name="qps")
                if kind == "off":
                    j0 = 2 * pidx
                    for t2 in range(2):
                        j = j0 + t2
                        nc.tensor.matmul(
                            ps[:, 512 * t2 : 512 * t2 + 512],
                            KTz[:, spo, sel, P * j : P * j + P],
                            QT[:, spo, q0 : q0 + 512],
                            start=True,
                            stop=True,
                        )
                    pt8 = pt8pool.tile([P, 1024], F8, tag="pt8", name="pt8")
                    if pidx % 4 == 1:
                        # fast-exp on DVE: y = s*FE_A + FE_B converted to
                        # int8, whose bits read as fp8e4m3 give ~exp(s/8)
                        nc.vector.tensor_scalar(
                            pt8[:].bitcast(I8), ps[:], FE_A, FE_B,
                            mybir.AluOpType.mult, mybir.AluOpType.add)
                    else:
                        nc.scalar.activation(pt8[:], ps[:], EXP, scale=0.125)
                    return [("dr", pt8, j0)]
                if kind == "dA" and c >= 1:
                    # fp8 DoubleRow dA (rows attend >=512 keys): j1 sits
                    # tq-aligned at cols 640:1024; 512:640 stay pool-zeroed
                    j0, j1 = 4 * c, 4 * c + 1
                    nc.tensor.matmul(
                        ps[:, 0:512],
                        KTz[:, spo, sel, P * j0 : P * j0 + P],
                        QT[:, spo, q0 : q0 + 512],
                        start=True,
                        stop=True,
                    )
                    nc.tensor.matmul(
                        ps[:, 640:1024],
                        KTz[:, spo, sel, P * j1 : P * j1 + P],
                        QT[:, spo, q0 + 128 : q0 + 512],
                        start=True,
                        stop=True,
                    )
                    pt8 = pt8dpool.tile([P, 1024], F8, tag="pt8d",
                                        name="pt8d")
                    nc.scalar.activation(pt8[:, 0:512], ps[:, 0:512], EXP,
                                         scale=0.125)
                    nc.scalar.activation(pt8[:, 640:1024], ps[:, 640:1024],
                                         EXP, scale=0.125)
                    masks.append(("aff8", pt8[:, 0:P]))
                    masks.append(("aff8", pt8[:, 640 : 640 + P]))
                    return [("dr", pt8, j0)]
                pt = ptpool.tile([P, 1024], BF16, tag="pt", name="pt")
                if kind == "dA":
                    j0, j1 = 4 * c, 4 * c + 1
                    nc.tensor.matmul(
                        ps[:, 0:512],
                        KTz[:, spo, sel, P * j0 : P * j0 + P],
                        QT[:, spo, q0 : q0 + 512],
                        start=True,
                        stop=True,
                    )
                    nc.tensor.matmul(
                        ps[:, 512:896],
                        KTz[:, spo, sel, P * j1 : P * j1 + P],
                        QT[:, spo, q0 + 128 : q0 + 512],
                        start=True,
                        stop=True,
                    )
                    nc.scalar.activation(pt[:, 0:896], ps[:, 0:896], EXP,
                                         scale=0.125)
                    # mask regions: block j0 cols [0,128) at off 0; block j1
                    # cols [128,256) at off 512+0
                    mv = pt[:].rearrange("p (a b) -> p a b", a=2)[:, :, 0:P]
                    masks.append(("mul", mv, tri2[:]))
                    return [("mm", pt, j0, 0, 0, 512),
                            ("mm", pt, j1, 512, 128, 384)]
                # dB
                j2, j3 = 4 * c + 2, 4 * c + 3
                ps = sm_ps.tile([P, 512], F32, tag="sm", name="dbps")
                nc.tensor.matmul(
                    ps[:, 0:256],
                    KTz[:, spo, sel, P * j2 : P * j2 + P],
                    QT[:, spo, q0 + 256 : q0 + 512],
                    start=True,
                    stop=True,
                )
                nc.tensor.matmul(
                    ps[:, 256:384],
                    KTz[:, spo, sel, P * j3 : P * j3 + P],
                    QT[:, spo, q0 + 384 : q0 + 512],
                    start=True,
                    stop=True,
                )
                nc.scalar.activation(pt[:, 0:384], ps[:, 0:384], EXP,
                                     scale=0.125)
                # mask regions: block j2 cols [256,384) at off 0; block j3
                # cols [384,512) at off 256
                mv = pt[:].rearrange("p (a b) -> p a b", a=4)[:, 0:2, 0:P]
                masks.append(("mul", mv, tri2[:]))
                return [("mm", pt, j2, 0, 256, 256),
                        ("mm", pt, j3, 256, 384, 128)]

            # ---- per-step state ----
            pend_pv = None  # (c, h, entries) awaiting PV in the next step
            masks = []  # deferred diagonal mask multiplies

            class PvStepper:
                """Emits the PV accumulation for one (c,h) step, one entry at
                a time, so the matmuls can interleave between the next step's
                QK pairs (keeping the PE fed while ScalarE/DVE drain exps)."""

                def __init__(self, c, h, entries):
                    self.c, self.h, self.entries = c, h, entries
                    self.i = 0
                    self.pv = pv_ps.tile([D + 1, 512], F32, tag="pv",
                                         name="pv")

                def step(self, k=1):
                    h, n = self.h, len(self.entries)
                    while k > 0 and self.i < n:
                        e = self.entries[self.i]
                        if e[0] == "dr":
                            _, pt8, j = e
                            nc.tensor.matmul(
                                self.pv[:, 0:512],
                                VE8[:, j : j + 2, h, 0 : D + 1],
                                pt8[:].rearrange("p (a n) -> p a n", a=2),
                                start=(self.i == 0),
                                stop=(self.i == n - 1),
                                perf_mode=DRM,
                                skip_group_check=True,
                            )
                        else:
                            _, pt, j, off, st, w = e
                            nc.tensor.matmul(
                                self.pv[:, st : st + w],
                                VE[:, j, h, :],
                                pt[:, off : off + w],
                                start=(self.i == 0),
                                stop=(self.i == n - 1),
                                skip_group_check=True,
                            )
                        self.i += 1
                        k -= 1

                def finish(self, units, quota, split=False):
                    n = len(self.entries)
                    while self.i < n:
                        if self.i % 2 == 1 and units and quota:
                            units.pop(0)()
                            quota -= 1
                        self.step()
                    while units and quota:
                        units.pop(0)()
                        quota -= 1
                    ot = otpool.tile([D + 1, 512], F32, tag="ot", name="ot")
                    if split:
                        # tail: stream the first half while casting the rest
                        nc.vector.tensor_copy(ot[:, 0:256], self.pv[:, 0:256])
                        nc.sync.dma_start(out_v[self.c, self.h, :, 0:256],
                                          ot[:, 0:256])
                        nc.vector.tensor_copy(ot[:, 256:512],
                                              self.pv[:, 256:512])
                        nc.sync.dma_start(out_v[self.c, self.h, :, 256:512],
                                          ot[:, 256:512])
                    else:
                        nc.vector.tensor_copy(ot[:], self.pv[:])
                        nc.sync.dma_start(out_v[self.c, self.h], ot[:])

            # ---- prologue: slice-0 projections ----
            for m in range(MCH):
                unit_qk("q", 0, m)
            for m in range(MCH):
                unit_qk("k", 0, m)
            for r in range(4):
                unit_v(0, r)

            # ---- main steps ----
            units = []
            for c in range(TC):
                if c + 1 < TC:
                    for m in range(MCH):
                        units.append(
                            lambda m=m, n=c + 1: unit_qk("q", n, m))
                        units.append(
                            lambda m=m, n=c + 1: unit_qk("k", n, m))
                    for r in range(4):
                        units.append(lambda r=r, n=c + 1: unit_v(n, r))
                for h in range(HLOC):
                    if c == TC - 1:
                        quota = 0
                    else:
                        quota = len(units) if h == HLOC - 1 else 2
                    kinds = [("off", p) for p in range(2 * c)]
                    kinds += [("dA", 0), ("dB", 0)]
                    stepper = PvStepper(*pend_pv) if pend_pv else None
                    entries = []
                    for np_, (kind, pidx) in enumerate(kinds):
                        entries += emit_qk_pair(c, h, kind, pidx)
                        if np_ == 1 and units and quota:
                            units.pop(0)()
                            quota -= 1
                    if stepper:
                        stepper.finish(units, quota)
                    # deferred diagonal masks flush after the PV cast (on
                    # gpsimd, which is otherwise idle mid-kernel)
                    for item in masks:
                        if item[0] == "mul":
                            nc.gpsimd.tensor_mul(item[1], item[1], item[2])
                        else:
                            nc.gpsimd.affine_select(
                                out=item[1], in_=item[1],
                                compare_op=mybir.AluOpType.is_ge,
                                fill=0.0, base=0, pattern=[[1, P]],
                                channel_multiplier=-1)
                    masks.clear()
                    pend_pv = (c, h, entries)

            # ---- drain ----
            st_ = PvStepper(*pend_pv)
            st_.finish([], 0, split=True)

    nc.compile()
    return nc


_NC_CACHE = {}


def _get_nc(T):
    if T not in _NC_CACHE:
        _NC_CACHE[T] = build(T)
    return _NC_CACHE[T]


def kernel(inputs_q, inputs_kv, Wq, Wk, Wv):
    inputs_q = np.asarray(inputs_q, dtype=np.float32)
    inputs_kv = np.asarray(inputs_kv, dtype=np.float32)
    Wq = np.asarray(Wq, dtype=np.float32)
    Wk = np.asarray(Wk, dtype=np.float32)
    Wv = np.asarray(Wv, dtype=np.float32)
    T = inputs_q.shape[1]
    TC = T // 512

    bf = ml_dtypes.bfloat16
    f8 = ml_dtypes.float8_e4m3
    in_maps = []
    for c in range(N_CORES):
        b, g = c // 2, c % 2
        sl = slice(g * DLOC, (g + 1) * DLOC)
        xqt = np.ascontiguousarray(inputs_q[b].T)
        xkvt = np.ascontiguousarray(inputs_kv[b].T)
        in_maps.append(
            {
                "xqT": xqt[:, 0:512].astype(bf),
                "xkvT": xkvt[:, 0:512].astype(bf),
                "xqT8": xqt[:, 512:].astype(f8),
                "xkvT8": xkvt[:, 512:].astype(f8),
                "wqT": np.ascontiguousarray(Wq[sl].T).astype(bf),
                "wkT": np.ascontiguousarray(Wk[sl].T).astype(bf),
                "wvT": np.ascontiguousarray(Wv[sl].T).astype(bf),
                "wqT8": np.ascontiguousarray(Wq[sl].T).astype(f8),
                "wkT8": np.ascontiguousarray(Wk[sl].T).astype(f8),
                "wvT8": np.ascontiguousarray(Wv[sl].T).astype(f8),
            }
        )

    nc = _get_nc(T)
    trace = bool(int(os.environ.get("KERNEL_TRACE", "0")))
    res = run_bass_kernel_spmd(
        nc, in_maps, core_ids=list(range(N_CORES)), trace=trace
    )
    if trace:
        kernel.last_result = res

    full = np.empty((B, T, E), np.float32)
    for c in range(N_CORES):
        b, g = c // 2, c % 2
        ot = res.results[c]["out"]  # [TC, HLOC, D+1, 512]
        o = ot[:, :, 0:D, :] / ot[:, :, D : D + 1, :]
        # [TC, HLOC, D, 512] -> [TC, 512, HLOC, D] -> [T, DLOC]
        o = o.transpose(0, 3, 1, 2).reshape(T, DLOC)
        full[b, :, g * DLOC : (g + 1) * DLOC] = o
    return full
